# revision 60
# baseline (speedup 1.0000x reference)
"""Trainium2 Bass kernel for the attention-LSTM decoder (fp8/fp16 rewrite).

Computation (per batch b; all T positions share the same (h0, c0) state):
  h0 = tanh(eh @ bridge_hW.T);  c0 = tanh(ec @ bridge_cW.T)
  energy = tanh(enc @ key_W.T + h0 @ query_W.T);  scores = energy . energy_W
  alphas = softmax(scores);  ctx = alphas @ enc          (mask == all-ones)
  gates = emb[tok] @ W_ih[:, :E].T + (ctx @ W_ih[:, E:].T + h0 @ W_hh.T + b)
  c = sig(f)*c0 + sig(i)*tanh(g);  h = sig(o)*tanh(c)
  out = emb[tok] @ pre_W[:, :E].T + h @ pre_W[:, E:E+H].T + ctx @ pre_W[:, E+H:].T

Design notes (v2):
  - Data-parallel over batch: 16 batches per core, processed as 8 pairs.
  - The token-path GEMMs (gates, output emb-part) run fp8e4 DoubleRow with
    x16 operand scales; embeddings are fetched by gpsimd gather_transpose
    straight into the fp8 DoubleRow feature-major layout.
  - keyproj runs fp8 DoubleRow with one residual level on each operand
    (e1k1 + e2k1 + e1k2), giving ~fp16 accuracy at fp8-DR speed.
  - The softmax/context/state paths stay fp16 (fp8 there compounds above
    the 2e-2 budget).
  - Per-batch gate/query biases are folded into PSUM with one-hot fp16
    matmuls so activations run bias-free and full-width; softmax is
    unstabilized (scores bounded ~|5|) and unnormalized -- the 1/sum(e)
    factors fold into the per-batch constants (gc, oc) in phase A2.
  - All values carry power-of-2 scales; the output returns x256 and the
    host divides.
"""

import numpy as np
import ml_dtypes
from contextlib import ExitStack

import concourse.bass as bass
import concourse.mybir as mybir
import concourse.tile as tile
from concourse import bacc, library_config
from concourse.bass_utils import run_bass_kernel_spmd
from concourse.masks import make_identity

FP32 = mybir.dt.float32
F16 = mybir.dt.float16
F8 = mybir.dt.float8e4
I16 = mybir.dt.int16
AF = mybir.ActivationFunctionType
OP = mybir.AluOpType
DR = mybir.MatmulPerfMode.DoubleRow

P = 128
H = 512
E = 256
TWOH = 1024
FOURH = 2048
S = 256
T = 256
V = 10000
N_CORES = 8
B_FULL = 128
NB = 16           # batches per core
NPAIR = NB // 2   # batch pairs == 512-token tiles

f8np = ml_dtypes.float8_e4m3


def build_kernel(nc, iters=1, fold_gates=True, fold_qproj=True, A2W_AT=7,
                 res_both=True):
    ntok = NB * T

    dt = lambda name, shape, dtype: nc.dram_tensor(
        name, shape, dtype, kind="ExternalInput")

    encT8_d = dt("encT8", [NPAIR * P, 8 * 512], F8)
    encT28_d = dt("encT28", [NPAIR * P, 8 * 512], F8)
    enc16_d = dt("enc16", [NB * P, 2 * TWOH], F16)
    emb16_d = dt("emb16", [V, E], F16)
    idx32_d = dt("idx32", [P, NPAIR * 4], mybir.dt.int32)
    sel16_d = dt("sel16", [NB, NPAIR * 512], F16)
    ehT16_d = dt("ehT16", [8 * P, NB], F16)
    ecT16_d = dt("ecT16", [8 * P, NB], F16)
    bhWT16_d = dt("bhWT16", [8 * P, H], F16)
    bcWT16_d = dt("bcWT16", [8 * P, H], F16)
    hb32_d = dt("hb32", [P, 4], FP32)
    cb32_d = dt("cb32", [P, 4], FP32)
    qWT16_d = dt("qWT16", [4 * P, H], F16)
    keyWT8_d = dt("keyWT8", [8 * P, H], F8)
    keyWT28_d = dt("keyWT28", [8 * P, H], F8)
    energyW16_d = dt("energyW16", [P, 4], F16)
    whh16_d = dt("whh16", [4 * P, FOURH], F16)
    wcx16_d = dt("wcx16", [8 * P, FOURH], F16)
    brow16_d = dt("brow16", [1, FOURH], F16)
    wihTembG8_d = dt("wihTembG8", [2 * P, FOURH], F8)
    preWTembG8_d = dt("preWTembG8", [2 * P, H], F8)
    preWTh16_d = dt("preWTh16", [4 * P, H], F16)
    preWTctx16_d = dt("preWTctx16", [8 * P, H], F16)
    out_d = nc.dram_tensor("out", [ntok, H], F16, kind="ExternalOutput")

    ch = lambda ap, k: ap.rearrange("(k p) n -> p k n", p=P)

    with ExitStack() as ctx:
        tc = ctx.enter_context(tile.TileContext(nc))
        if iters > 1:
            ctx.enter_context(tc.For_i(0, iters, 1))

        # ---------------- resident tiles ----------------
        const = ctx.enter_context(tc.tile_pool(name="const", bufs=1))
        identity16 = const.tile([P, P], F16)
        make_identity(nc, identity16[:])
        identity8 = const.tile([P, P], F8)
        nc.vector.tensor_copy(identity8[:], identity16[:])
        ones16f = const.tile([1, NB], F16)
        nc.vector.memset(ones16f[:], 1.0)
        onescol16 = const.tile([P, 1], F16)
        nc.vector.memset(onescol16[:], 1.0 / 256.0)


        idx_sb = const.tile([P, NPAIR * 4], mybir.dt.int32)
        nc.sync.dma_start(out=idx_sb[:], in_=idx32_d[:])
        sel_sb = const.tile([NB, NPAIR * 512], F16)
        nc.sync.dma_start(out=sel_sb[:], in_=sel16_d[:])
        energyW_sb = const.tile([P, 4], F16)
        nc.sync.dma_start(out=energyW_sb[:], in_=energyW16_d[:])
        hb_sb = const.tile([P, 4], FP32)
        nc.sync.dma_start(out=hb_sb[:], in_=hb32_d[:])
        cb_sb = const.tile([P, 4], FP32)
        nc.sync.dma_start(out=cb_sb[:], in_=cb32_d[:])

        keyWT_sb = const.tile([P, 8, H], F8)
        nc.sync.dma_start(out=keyWT_sb[:], in_=ch(keyWT8_d[:], 8))
        keyWT2_sb = const.tile([P, 8, H], F8)
        nc.sync.dma_start(out=keyWT2_sb[:], in_=ch(keyWT28_d[:], 8))

        # phase-B weights: tiles allocated here (resident) but their DMAs
        # are issued after the phase-A enc streams so prologue DMA stays thin
        wihTembG_sb = const.tile([P, 2, FOURH], F8)
        preWTembG_sb = const.tile([P, 2, H], F8)
        preWTh_sb = const.tile([P, 4, H], F16)
        gcT_sb = const.tile([P, 16, NB], FP32)

        # per-batch state, alive for the whole kernel
        state = ctx.enter_context(tc.tile_pool(name="state", bufs=1))
        h0T16_sb = state.tile([P, 4, NB], F16)
        c0T_sb = state.tile([P, 4, NB], FP32)
        qprojR_sb = state.tile([NB, H], F16)
        qprojT_sb = state.tile([P, 4, NB], FP32)
        eTall_sb = state.tile([P, 2, NB], F16)
        ctxT16_sb = state.tile([P, 8, NB], F16)
        rzall_sb = state.tile([NB, 1], FP32)
        gcR_sb = state.tile([NB, FOURH], F16)
        oc_sb = state.tile([NB, H], F16)
        # fp8 coarse+residual pairs for the PSUM fold matmuls (DoubleRow)
        sel8_sb = state.tile([NB, NPAIR * 512], F8)
        gc8p_sb = state.tile([NB, 2, FOURH], F8)
        oc8p_sb = state.tile([NB, 2, H], F8)
        qp8p_sb = state.tile([NB, 2, H], F8)
        embp = ctx.enter_context(tc.tile_pool(name="embp", bufs=NPAIR))

        # ---------------- setup: h0 / c0 / qproj ----------------
        with tc.tile_pool(name="setw", bufs=1) as setw, \
             tc.tile_pool(name="setps", bufs=2, space="PSUM") as setps:
            ehT_sb = setw.tile([P, 8, NB], F16)
            nc.sync.dma_start(out=ehT_sb[:], in_=ch(ehT16_d[:], 8))
            ecT_sb = setw.tile([P, 8, NB], F16)
            nc.sync.dma_start(out=ecT_sb[:], in_=ch(ecT16_d[:], 8))
            bhWT_sb = setw.tile([P, 8, H], F16)
            nc.sync.dma_start(out=bhWT_sb[:], in_=ch(bhWT16_d[:], 8))
            bcWT_sb = setw.tile([P, 8, H], F16)
            nc.sync.dma_start(out=bcWT_sb[:], in_=ch(bcWT16_d[:], 8))
            qWT_sb = setw.tile([P, 4, H], F16)
            nc.sync.dma_start(out=qWT_sb[:], in_=ch(qWT16_d[:], 4))

            for m in range(4):
                ps = setps.tile([P, NB], FP32, tag="ps")
                for k in range(8):
                    nc.tensor.matmul(
                        ps[:], bhWT_sb[:, k, m * P:(m + 1) * P],
                        ehT_sb[:, k, :], start=(k == 0), stop=(k == 7))
                nc.scalar.activation(h0T16_sb[:, m, :], ps[:], AF.Tanh,
                                     bias=hb_sb[:, m:m + 1])
            for m in range(4):
                ps = setps.tile([P, NB], FP32, tag="ps")
                for k in range(8):
                    nc.tensor.matmul(
                        ps[:], bcWT_sb[:, k, m * P:(m + 1) * P],
                        ecT_sb[:, k, :], start=(k == 0), stop=(k == 7))
                nc.scalar.activation(c0T_sb[:, m, :], ps[:], AF.Tanh,
                                     bias=cb_sb[:, m:m + 1])
            # qprojR [b, H] = h0 @ query_W.T  (x16 via qWT16 scaling)
            qps = setps.tile([NB, H], FP32, tag="qps")
            for k in range(4):
                nc.tensor.matmul(qps[:], h0T16_sb[:, k, :], qWT_sb[:, k, :],
                                 start=(k == 0), stop=(k == 3))
            if fold_qproj:
                nc.vector.tensor_copy(qprojR_sb[:], qps[:])
                # pairs stored at 1/16 of the PSUM scale; sel8 carries x16
                nc.vector.tensor_scalar_mul(sel8_sb[:], sel_sb[:], 16.0)
                nc.vector.tensor_scalar_mul(qp8p_sb[:, 0, :], qps[:],
                                            1.0 / 16.0)
                nc.vector.scalar_tensor_tensor(
                    out=qp8p_sb[:, 1, :], in0=qps[:], scalar=1.0 / 16.0,
                    in1=qp8p_sb[:, 0, :], op0=OP.mult, op1=OP.subtract)
            else:
                # transpose to per-(chunk, batch) bias columns, true scale
                qp16 = setw.tile([NB, H], F16)
                nc.vector.tensor_copy(qp16[:], qps[:])
                for m in range(4):
                    tq = setps.tile([P, NB], F16, tag="tq")
                    nc.tensor.transpose(tq[:], qp16[0:NB, m * P:(m + 1) * P],
                                        identity16[0:NB, 0:NB])
                    nc.vector.tensor_scalar_mul(qprojT_sb[:, m, :], tq[:],
                                                1.0 / 16.0)

        # embedding gathers (idx-only dependence; Pool-driven indirect
        # DMAs stream during setup/phase A). Token-major [tok, E].
        emb_tiles = []
        for gi in range(NPAIR):
            ge = embp.tile([P, 4, E], F16, tag="ge")
            for j in range(4):
                nc.gpsimd.indirect_dma_start(
                    out=ge[:, j, :], out_offset=None,
                    in_=emb16_d[:],
                    in_offset=bass.IndirectOffsetOnAxis(
                        ap=idx_sb[:, gi * 4 + j:gi * 4 + j + 1], axis=0))
            emb_tiles.append(ge)
        embT_tiles = []
        embTp = ctx.enter_context(tc.tile_pool(name="embTp", bufs=NPAIR))

        # ---------------- phase A: attention ----------------
        a2w = ctx.enter_context(tc.tile_pool(name="a2w", bufs=1))
        whh_sb = a2w.tile([P, 4, FOURH], F16)
        wcx_sb = a2w.tile([P, 8, FOURH], F16)
        brow_sb = a2w.tile([1, FOURH], F16)
        pwctx_sb = a2w.tile([P, 8, H], F16)
        with tc.tile_pool(name="encTp", bufs=2) as encTp, \
             tc.tile_pool(name="encT2p", bufs=2) as encT2p, \
             tc.tile_pool(name="encp", bufs=3) as encp, \
             tc.tile_pool(name="enerp", bufs=2) as enerp, \
             tc.tile_pool(name="erow", bufs=4) as erow, \
             tc.tile_pool(name="pspk", bufs=3, space="PSUM") as pspk, \
             tc.tile_pool(name="pssc", bufs=1, space="PSUM") as pssc, \
             tc.tile_pool(name="psct", bufs=1, space="PSUM") as psct:
            for i in range(NPAIR):
                if i == A2W_AT:
                    # A2 weights: queued after the enc streams of this pair
                    nc.sync.dma_start(out=whh_sb[:], in_=ch(whh16_d[:], 4))
                    nc.sync.dma_start(out=wcx_sb[:], in_=ch(wcx16_d[:], 8))
                    nc.sync.dma_start(out=brow_sb[:], in_=brow16_d[:])
                    nc.sync.dma_start(out=pwctx_sb[:],
                                      in_=ch(preWTctx16_d[:], 8))
                encTt = encTp.tile([P, 8, 512], F8, tag="encT")
                nc.sync.dma_start(
                    out=encTt[:],
                    in_=encT8_d[i * P:(i + 1) * P, :].rearrange(
                        "p (k n) -> p k n", k=8))
                if res_both:
                    encT2t = encT2p.tile([P, 8, 512], F8, tag="encT2")
                    nc.sync.dma_start(
                        out=encT2t[:],
                        in_=encT28_d[i * P:(i + 1) * P, :].rearrange(
                            "p (k n) -> p k n", k=8))
                enc_x = []
                for x in range(2):
                    et = encp.tile([P, 2, TWOH], F16, tag="enc")
                    b = 2 * i + x
                    nc.sync.dma_start(
                        out=et[:],
                        in_=enc16_d[b * P:(b + 1) * P, :].rearrange(
                            "p (c n) -> p c n", c=2))
                    enc_x.append(et)

                ener = enerp.tile([P, 4, H], F16, tag="ener")
                for h2 in range(2):
                    pk = pspk.tile([P, TWOH], FP32, tag="pk")
                    for mc in range(2):
                        m = 2 * h2 + mc
                        win = pk[:, mc * H:(mc + 1) * H]
                        mw = slice(m * P, (m + 1) * P)
                        passes = [(keyWT_sb, encTt), (keyWT2_sb, encTt)]
                        if res_both:
                            passes.insert(1, (keyWT_sb, encT2t))
                        for pi, (kw, et) in enumerate(passes):
                            for kp in range(4):
                                kk = slice(2 * kp, 2 * kp + 2)
                                last = (not fold_qproj
                                        and pi == len(passes) - 1
                                        and kp == 3)
                                nc.tensor.matmul(win, kw[:, kk, mw],
                                                 et[:, kk, :],
                                                 start=(pi == 0 and kp == 0),
                                                 stop=last, perf_mode=DR)
                        if fold_qproj:
                            s8 = sel8_sb[0:NB, i * 512:(i + 1) * 512]
                            s8p = bass.AP(s8.tensor, s8.offset,
                                          [s8.ap[0], [0, 2], s8.ap[1]])
                            nc.tensor.matmul(
                                win, qp8p_sb[0:NB, 0:2, mw], s8p,
                                start=False, stop=True, perf_mode=DR)
                    if fold_qproj:
                        nc.scalar.activation(
                            ener[:, 2 * h2:2 * h2 + 2, :],
                            pk[:].rearrange("p (c n) -> p c n", c=2),
                            AF.Tanh, scale=1.0 / 16.0)
                    else:
                        for mc in range(2):
                            m = 2 * h2 + mc
                            for x in range(2):
                                b = 2 * i + x
                                nc.scalar.activation(
                                    ener[:, m, x * S:(x + 1) * S],
                                    pk[:, mc * H + x * S:mc * H + (x + 1) * S],
                                    AF.Tanh, scale=1.0 / 16.0,
                                    bias=qprojT_sb[:, m, b:b + 1])

                ctall = psct.tile([P, 20], FP32, tag="ct")
                for x in range(2):
                    sc = pssc.tile([1, S], FP32, tag="sc")
                    for m in range(4):
                        nc.tensor.matmul(
                            sc[:], energyW_sb[:, m:m + 1],
                            ener[:, m, x * S:(x + 1) * S],
                            start=(m == 0), stop=(m == 3))
                    # scores are bounded (|s| < ~6) for this model: softmax
                    # runs unstabilized, unnormalized.
                    e16 = erow.tile([1, S], F16, tag="e16")
                    nc.scalar.activation(e16[:], sc[:], AF.Exp,
                                         scale=1.0 / 16.0)
                    for c in range(2):
                        nc.tensor.matmul(ctall[:, c * 2 + x:c * 2 + x + 1],
                                         e16[0:1, c * P:(c + 1) * P],
                                         ones16f[0:1, 0:1], start=True,
                                         stop=True)
                nc.vector.tensor_copy(
                    eTall_sb[:, :, 2 * i:2 * i + 2],
                    ctall[:, 0:4].rearrange("p (c x) -> p c x", c=2))
                for x in range(2):
                    b = 2 * i + x
                    for c8 in range(8):
                        for c in range(2):
                            nc.tensor.matmul(
                                ctall[:, 4 + c8 * 2 + x:5 + c8 * 2 + x],
                                enc_x[x][:, c, c8 * P:(c8 + 1) * P],
                                eTall_sb[:, c, b:b + 1],
                                start=(c == 0), stop=(c == 1))
                # ctxT16 = sum(e * enc) / 16 = Z*ctx/16 (unnormalized Z~500)
                nc.vector.tensor_scalar_mul(
                    ctxT16_sb[:, :, 2 * i:2 * i + 2],
                    ctall[:, 4:20].rearrange("p (c x) -> p c x", c=8),
                    1.0 / 16.0)


        # transpose gathered [tok, E] -> DR layout [e%128, e//128, tok];
        # runs on the PE during the A2 weight-DMA window
        with tc.tile_pool(name="pstr", bufs=2, space="PSUM") as pstr:
            for ti in range(NPAIR):
                embT = embTp.tile([P, 2, 512], F8, tag="embT")
                tp = pstr.tile([P, 2, 512], F16, tag="tp")
                for j in range(4):
                    for e2 in range(2):
                        nc.tensor.transpose(
                            tp[:, e2, j * P:(j + 1) * P],
                            emb_tiles[ti][:, j, e2 * P:(e2 + 1) * P],
                            identity16[:])
                nc.scalar.copy(embT[:].rearrange("p c n -> p (c n)"),
                               tp[:].rearrange("p c n -> p (c n)"))
                embT_tiles.append(embT)

        # ---------------- A2: fold 1/Z, per-batch constants ----------------
        with tc.tile_pool(name="psa2", bufs=1, space="PSUM") as psa2, \
             tc.tile_pool(name="psgc", bufs=1, space="PSUM") as psgc:

            zps = psa2.tile([NB, 1], FP32, tag="z")
            for c in range(2):
                nc.tensor.matmul(zps[:], eTall_sb[:, c, :], onescol16[:, 0:1],
                                 start=(c == 0), stop=(c == 1))
            nc.vector.reciprocal(rzall_sb[:], zps[:])  # = 256 / sum(e)

            for half in range(2):
                gch = psgc.tile([NB, FOURH // 2], FP32, tag="gch")
                gcx = psgc.tile([NB, FOURH // 2], FP32, tag="gcx")
                gch16 = a2w.tile([NB, FOURH // 2], F16, tag="gch16")
                for nw2 in range(2):
                    nw = 2 * half + nw2
                    win = slice(nw * H, (nw + 1) * H)
                    pw = slice(nw2 * H, (nw2 + 1) * H)
                    for k in range(4):
                        nc.tensor.matmul(gch[:, pw], h0T16_sb[:, k, :],
                                         whh_sb[:, k, win],
                                         start=(k == 0), stop=False)
                    nc.tensor.matmul(gch[:, pw], ones16f[0:1, :],
                                     brow_sb[0:1, win], start=False, stop=True)
                    for k in range(8):
                        nc.tensor.matmul(gcx[:, pw], ctxT16_sb[:, k, :],
                                         wcx_sb[:, k, win],
                                         start=(k == 0), stop=(k == 7))
                nc.vector.tensor_copy(gch16[:], gch[:])
                nc.vector.scalar_tensor_tensor(
                    out=gcR_sb[:, half * TWOH:(half + 1) * TWOH],
                    in0=gcx[:], scalar=rzall_sb[:, 0:1],
                    in1=gch16[:], op0=OP.mult, op1=OP.add)

            ocps = psa2.tile([NB, H], FP32, tag="oc")
            for k in range(8):
                nc.tensor.matmul(ocps[:], ctxT16_sb[:, k, :], pwctx_sb[:, k, :],
                                 start=(k == 0), stop=(k == 7))
            nc.vector.tensor_scalar_mul(oc_sb[:], ocps[:], rzall_sb[:, 0:1])
            if fold_gates:
                nc.vector.tensor_scalar_mul(gc8p_sb[:, 0, :], gcR_sb[:],
                                            1.0 / 16.0)
                nc.vector.scalar_tensor_tensor(
                    out=gc8p_sb[:, 1, :], in0=gcR_sb[:], scalar=1.0 / 16.0,
                    in1=gc8p_sb[:, 0, :], op0=OP.mult, op1=OP.subtract)
            nc.vector.tensor_copy(oc8p_sb[:, 0, :], oc_sb[:])
            nc.vector.tensor_tensor(out=oc8p_sb[:, 1, :], in0=oc_sb[:],
                                    in1=oc8p_sb[:, 0, :], op=OP.subtract)

            if not fold_gates:
                # transpose gcR rows into per-(chunk, batch) bias columns,
                # true scale (gcR is x256)
                for mg in range(16):
                    tg = psa2.tile([P, NB], F16, tag="tg")
                    nc.tensor.transpose(
                        tg[:], gcR_sb[0:NB, mg * P:(mg + 1) * P],
                        identity16[0:NB, 0:NB])
                    nc.vector.tensor_scalar_mul(gcT_sb[:, mg, :], tg[:],
                                                1.0 / 256.0)

            # phase-B weights (DMAs deferred here so the phase-A enc
            # streams own the DMA engines during phase A)
            nc.sync.dma_start(out=wihTembG_sb[:], in_=ch(wihTembG8_d[:], 2))
            nc.sync.dma_start(out=preWTembG_sb[:], in_=ch(preWTembG8_d[:], 2))
            nc.sync.dma_start(out=preWTh_sb[:], in_=ch(preWTh16_d[:], 4))

        # ---------------- phase B: gates, LSTM, projection ----------------
        # wave order: i, g, f, o (mg = gate*4 + hs; torch order i,f,g,o)
        WAVES = [(0, AF.Sigmoid), (2, AF.Tanh), (1, AF.Sigmoid),
                 (3, AF.Sigmoid)]
        with tc.tile_pool(name="gmega", bufs=2) as gmega, \
             tc.tile_pool(name="lstm", bufs=2) as lstm, \
             tc.tile_pool(name="outp", bufs=2) as outp, \
             tc.tile_pool(name="pswv", bufs=3, space="PSUM") as pswv, \
             tc.tile_pool(name="pspo", bufs=2, space="PSUM") as pspo:
            pend = None
            for i in range(NPAIR + 1):
              if i < NPAIR:
                embT = embT_tiles[i]
                mega = {}
                for g, fn in WAVES:
                    dst = gmega.tile([P, 4, 512], F16, tag=f"m{g}")
                    mega[g] = dst
                    for hp in range(2):
                        wv = pswv.tile([P, TWOH], FP32, tag="wv")
                        for mc in range(2):
                            hs = 2 * hp + mc
                            mg = g * 4 + hs
                            win = wv[:, mc * H:(mc + 1) * H]
                            nc.tensor.matmul(
                                win, wihTembG_sb[:, 0:2, mg * P:(mg + 1) * P],
                                embT[:, 0:2, :], start=True,
                                stop=not fold_gates, perf_mode=DR)
                            if fold_gates:
                                s8 = sel8_sb[0:NB, i * 512:(i + 1) * 512]
                                s8p = bass.AP(s8.tensor, s8.offset,
                                              [s8.ap[0], [0, 2], s8.ap[1]])
                                nc.tensor.matmul(
                                    win,
                                    gc8p_sb[0:NB, 0:2, mg * P:(mg + 1) * P],
                                    s8p, start=False, stop=True, perf_mode=DR)
                        if fold_gates:
                            nc.scalar.activation(
                                dst[:, 2 * hp:2 * hp + 2, :],
                                wv[:].rearrange("p (c n) -> p c n", c=2),
                                fn, scale=1.0 / 256.0)
                        else:
                            for mc in range(2):
                                hs = 2 * hp + mc
                                mg = g * 4 + hs
                                for x in range(2):
                                    b = 2 * i + x
                                    nc.scalar.activation(
                                        dst[:, hs, x * S:(x + 1) * S],
                                        wv[:, mc * H + x * S:
                                           mc * H + (x + 1) * S],
                                        fn, scale=1.0 / 256.0,
                                        bias=gcT_sb[:, mg, b:b + 1])

                sI, tG, sF, sO = mega[0], mega[2], mega[1], mega[3]
                tmp = lstm.tile([P, FOURH], F16, tag="tmp")
                nc.vector.tensor_tensor(out=tmp[:],
                                        in0=sI[:].rearrange("p c n -> p (c n)"),
                                        in1=tG[:].rearrange("p c n -> p (c n)"),
                                        op=OP.mult)
                cc = lstm.tile([P, 4, 2, S], F16, tag="cc")
                for hs in range(4):
                    for x in range(2):
                        b = 2 * i + x
                        nc.vector.scalar_tensor_tensor(
                            out=cc[:, hs, x, :],
                            in0=sF[:, hs, x * S:(x + 1) * S],
                            scalar=c0T_sb[:, hs, b:b + 1],
                            in1=tmp[:, hs * 512 + x * S:
                                    hs * 512 + (x + 1) * S],
                            op0=OP.mult, op1=OP.add)
                tc16 = lstm.tile([P, FOURH], F16, tag="tc")
                nc.scalar.activation(tc16[:],
                                     cc[:].rearrange("p h x s -> p (h x s)"),
                                     AF.Tanh)
                h16 = lstm.tile([P, 4, 512], F16, tag="h16")
                nc.vector.scalar_tensor_tensor(
                    out=h16[:].rearrange("p c n -> p (c n)"),
                    in0=sO[:].rearrange("p c n -> p (c n)"), scalar=16.0,
                    in1=tc16[:], op0=OP.mult, op1=OP.mult)

                pend_new = (i, embT, h16)
              else:
                pend_new = None
              if pend is not None:
                pi_, embT_, h16_ = pend
                for tci in range(4):
                    x, tokw = tci // 2, tci % 2
                    b = 2 * pi_ + x
                    tw = slice(x * S + tokw * P, x * S + (tokw + 1) * P)
                    po = pspo.tile([P, H], FP32, tag="po")
                    nc.tensor.matmul(po[:],
                                     embT_[:, 0:2, tci * P:(tci + 1) * P],
                                     preWTembG_sb[:, 0:2, :],
                                     start=True, stop=False, perf_mode=DR)
                    for hs in range(4):
                        nc.tensor.matmul(po[:], h16_[:, hs, tw],
                                         preWTh_sb[:, hs, :],
                                         start=False, stop=False)
                    idc = identity8[0:NB, b:b + 1]
                    idp = bass.AP(idc.tensor, idc.offset,
                                  [idc.ap[0], [0, 2], [0, P]])
                    nc.tensor.matmul(po[:], idp, oc8p_sb[0:NB, 0:2, :],
                                     start=False, stop=True, perf_mode=DR)
                    o16 = outp.tile([P, H], F16, tag="o16")
                    nc.vector.tensor_copy(o16[:], po[:])
                    nc.sync.dma_start(
                        out=out_d[pi_ * 512 + tci * P:
                                  pi_ * 512 + (tci + 1) * P, :],
                        in_=o16[:])
              pend = pend_new
    return nc


# ---------------------------------------------------------------------------
# host side
# ---------------------------------------------------------------------------

def _to8(x):
    return np.clip(np.asarray(x, np.float32), -240.0, 240.0).astype(f8np)


def _chunkmajor(v, chunks):
    return np.ascontiguousarray(v.reshape(chunks, P).T).astype(np.float32)


def prep_inputs(inputs, n_cores=N_CORES):
    f32 = lambda x: np.asarray(x, dtype=np.float32)
    f16c = lambda x: np.ascontiguousarray(f32(x)).astype(np.float16)
    tgt = np.asarray(inputs["tgt_seq"]).astype(np.int32)
    enc = f32(inputs["encoder_output"])
    eh = f32(inputs["encoder_hidden"])[0]
    ec = f32(inputs["encoder_cell"])[0]
    W_ih = f32(inputs["W_ih"])
    W_hh = f32(inputs["W_hh"])
    pre_W = f32(inputs["pre_W"])
    emb = f32(inputs["emb"])

    sel = np.zeros((NB, NPAIR, 2, 256), np.float16)
    for i in range(NPAIR):
        sel[2 * i, i, 0, :] = 1.0
        sel[2 * i + 1, i, 1, :] = 1.0

    k16 = f32(inputs["key_W"]).T * 16.0
    k1 = _to8(k16)
    k2 = _to8(k16 - k1.astype(np.float32))

    shared = dict(
        emb16=np.ascontiguousarray(emb * 16.0).astype(np.float16),
        sel16=np.ascontiguousarray(sel.reshape(NB, NPAIR * 512)),
        bhWT16=f16c(f32(inputs["bridge_hW"]).T),
        bcWT16=f16c(f32(inputs["bridge_cW"]).T),
        hb32=_chunkmajor(f32(inputs["bridge_hb"]), 4),
        cb32=_chunkmajor(f32(inputs["bridge_cb"]), 4),
        qWT16=f16c(f32(inputs["query_W"]).T * 16.0),
        keyWT8=k1,
        keyWT28=k2,
        energyW16=_chunkmajor(f32(inputs["energy_W"])[0], 4).astype(
            np.float16) * np.float16(16.0),
        whh16=f16c(W_hh.T * 256.0),
        wcx16=f16c(W_ih[:, E:].T * 16.0),
        brow16=f16c((f32(inputs["b_ih"]) + f32(inputs["b_hh"]))[None, :]
                    * 256.0),
        wihTembG8=_to8(W_ih[:, :E].T * 16.0),
        preWTembG8=_to8(pre_W[:, :E].T * 16.0),
        preWTh16=f16c(pre_W[:, E:E + H].T * 16.0),
        preWTctx16=f16c(pre_W[:, E + H:].T * 16.0),
    )

    in_maps = []
    for ci in range(n_cores):
        sl = slice(ci * NB, (ci + 1) * NB)
        enc_c = enc[sl]                                    # [NB, S, 2H]
        # encT8 [pair*128+p, k*512 + b*256 + s] = enc[2i+b, s, k*128+p]
        encT = enc_c.reshape(NPAIR, 2, S, 8, P)
        encT = np.transpose(encT, (0, 4, 3, 1, 2))          # [i, p, k, b, s]
        encT = np.ascontiguousarray(encT.reshape(NPAIR * P, 8 * 512))
        encT1 = _to8(encT)
        encT2 = _to8(encT - encT1.astype(np.float32))
        # enc16 [b*128+p, c*1024+d] = enc[b, c*128+p, d]
        enc16 = enc_c.reshape(NB, 2, P, TWOH)
        enc16 = np.transpose(enc16, (0, 2, 1, 3))           # [b, p, c, d]

        tgtc = tgt[sl]
        idx = np.zeros((P, NPAIR * 4), np.int32)
        for i in range(NPAIR):
            flat = tgtc[2 * i:2 * i + 2].reshape(512)
            idx[:, i * 4:(i + 1) * 4] = flat.reshape(4, P).T

        in_maps.append(dict(
            encT8=encT1,
            encT28=encT2,
            enc16=enc16.reshape(NB * P, 2 * TWOH).astype(np.float16),
            idx32=idx,
            ehT16=f16c(eh[sl].T),
            ecT16=f16c(ec[sl].T),
            **shared,
        ))
    return in_maps, NB


_CACHED = {}


FOLD_GATES = True
FOLD_QPROJ = True
RES_BOTH = False


def _get_nc(key=0, iters=1):
    k = (key, iters, FOLD_GATES, FOLD_QPROJ, RES_BOTH)
    if k not in _CACHED:
        nc = bacc.Bacc("TRN2", target_bir_lowering=False, debug=False)
        build_kernel(nc, iters=iters, fold_gates=FOLD_GATES,
                     fold_qproj=FOLD_QPROJ, res_both=RES_BOTH)
        nc.compile()
        _CACHED[k] = nc
    return _CACHED[k]


def kernel(**inputs):
    in_maps, _ = prep_inputs(inputs, N_CORES)
    nc = _get_nc()
    res = run_bass_kernel_spmd(nc, in_maps, list(range(N_CORES)))
    B = np.asarray(inputs["tgt_seq"]).shape[0]
    out = np.empty((B, T, H), dtype=np.float32)
    for i in range(N_CORES):
        o = res.results[i]["out"].astype(np.float32) / 256.0
        out[i * NB:(i + 1) * NB] = o.reshape(NB, T, H)
    return out


# revision 61
# speedup vs baseline: 1.2745x; 1.2745x over previous
"""Trainium2 Bass kernel for the attention-LSTM decoder (fp8/fp16 rewrite).

Computation (per batch b; all T positions share the same (h0, c0) state):
  h0 = tanh(eh @ bridge_hW.T);  c0 = tanh(ec @ bridge_cW.T)
  energy = tanh(enc @ key_W.T + h0 @ query_W.T);  scores = energy . energy_W
  alphas = softmax(scores);  ctx = alphas @ enc          (mask == all-ones)
  gates = emb[tok] @ W_ih[:, :E].T + (ctx @ W_ih[:, E:].T + h0 @ W_hh.T + b)
  c = sig(f)*c0 + sig(i)*tanh(g);  h = sig(o)*tanh(c)
  out = emb[tok] @ pre_W[:, :E].T + h @ pre_W[:, E:E+H].T + ctx @ pre_W[:, E+H:].T

Design notes (v2):
  - Data-parallel over batch: 16 batches per core, processed as 8 pairs.
  - The token-path GEMMs (gates, output emb-part) run fp8e4 DoubleRow with
    x16 operand scales; embeddings are fetched by gpsimd gather_transpose
    straight into the fp8 DoubleRow feature-major layout.
  - keyproj runs fp8 DoubleRow with one residual level on each operand
    (e1k1 + e2k1 + e1k2), giving ~fp16 accuracy at fp8-DR speed.
  - The softmax/context/state paths stay fp16 (fp8 there compounds above
    the 2e-2 budget).
  - Per-batch gate/query biases are folded into PSUM with one-hot fp16
    matmuls so activations run bias-free and full-width; softmax is
    unstabilized (scores bounded ~|5|) and unnormalized -- the 1/sum(e)
    factors fold into the per-batch constants (gc, oc) in phase A2.
  - All values carry power-of-2 scales; the output returns x256 and the
    host divides.
"""

import numpy as np
import ml_dtypes
from contextlib import ExitStack

import concourse.bass as bass
import concourse.mybir as mybir
import concourse.tile as tile
from concourse import bacc, library_config
from concourse.bass_utils import run_bass_kernel_spmd
from concourse.masks import make_identity

FP32 = mybir.dt.float32
F16 = mybir.dt.float16
F8 = mybir.dt.float8e4
I16 = mybir.dt.int16
AF = mybir.ActivationFunctionType
OP = mybir.AluOpType
DR = mybir.MatmulPerfMode.DoubleRow

P = 128
H = 512
E = 256
TWOH = 1024
FOURH = 2048
S = 256
T = 256
V = 10000
N_CORES = 8
B_FULL = 128
NB = 16           # batches per core
NPAIR = NB // 2   # batch pairs == 512-token tiles

f8np = ml_dtypes.float8_e4m3


def build_kernel(nc, iters=1, fold_gates=True, fold_qproj=True, A2W_AT=7,
                 res_both=True, phases="ab"):
    ntok = NB * T

    dt = lambda name, shape, dtype: nc.dram_tensor(
        name, shape, dtype, kind="ExternalInput")

    encT8_d = dt("encT8", [NPAIR * P, 8 * 512], F8)
    encT28_d = dt("encT28", [NPAIR * P, 8 * 512], F8)
    enc16_d = dt("enc16", [NB * P, 2 * TWOH], F16)
    emb16_d = dt("emb16", [V, E], F16)
    idx32_d = dt("idx32", [P, NPAIR * 4], mybir.dt.int32)
    sel16_d = dt("sel16", [NB, NPAIR * 512], F16)
    ehT16_d = dt("ehT16", [8 * P, NB], F16)
    ecT16_d = dt("ecT16", [8 * P, NB], F16)
    bhWT16_d = dt("bhWT16", [8 * P, H], F16)
    bcWT16_d = dt("bcWT16", [8 * P, H], F16)
    hb32_d = dt("hb32", [P, 4], FP32)
    cb32_d = dt("cb32", [P, 4], FP32)
    qWT16_d = dt("qWT16", [4 * P, H], F16)
    keyWT8_d = dt("keyWT8", [8 * P, H], F8)
    keyWT28_d = dt("keyWT28", [8 * P, H], F8)
    energyW16_d = dt("energyW16", [P, 4], F16)
    whh16_d = dt("whh16", [4 * P, FOURH], F16)
    wcx16_d = dt("wcx16", [8 * P, FOURH], F16)
    brow16_d = dt("brow16", [1, FOURH], F16)
    wihTembG8_d = dt("wihTembG8", [2 * P, FOURH], F8)
    preWTembG8_d = dt("preWTembG8", [2 * P, H], F8)
    preWTh16_d = dt("preWTh16", [4 * P, H], F16)
    preWTctx16_d = dt("preWTctx16", [8 * P, H], F16)
    out_d = nc.dram_tensor("out", [ntok, H], F16, kind="ExternalOutput")

    ch = lambda ap, k: ap.rearrange("(k p) n -> p k n", p=P)

    with ExitStack() as ctx:
        tc = ctx.enter_context(tile.TileContext(nc))
        if iters > 1:
            ctx.enter_context(tc.For_i(0, iters, 1))

        # ---------------- resident tiles ----------------
        const = ctx.enter_context(tc.tile_pool(name="const", bufs=1))
        identity16 = const.tile([P, P], F16)
        make_identity(nc, identity16[:])
        identity8 = const.tile([P, P], F8)
        nc.vector.tensor_copy(identity8[:], identity16[:])
        ones16f = const.tile([1, NB], F16)
        nc.vector.memset(ones16f[:], 1.0)
        onescol16 = const.tile([P, 1], F16)
        nc.vector.memset(onescol16[:], 1.0 / 256.0)


        idx_sb = const.tile([P, NPAIR * 4], mybir.dt.int32)
        nc.sync.dma_start(out=idx_sb[:], in_=idx32_d[:])
        sel_sb = const.tile([NB, NPAIR * 512], F16)
        nc.sync.dma_start(out=sel_sb[:], in_=sel16_d[:])
        energyW_sb = const.tile([P, 4], F16)
        nc.sync.dma_start(out=energyW_sb[:], in_=energyW16_d[:])
        hb_sb = const.tile([P, 4], FP32)
        nc.sync.dma_start(out=hb_sb[:], in_=hb32_d[:])
        cb_sb = const.tile([P, 4], FP32)
        nc.sync.dma_start(out=cb_sb[:], in_=cb32_d[:])

        keyWT_sb = const.tile([P, 8, H], F8)
        nc.sync.dma_start(out=keyWT_sb[:], in_=ch(keyWT8_d[:], 8))
        keyWT2_sb = const.tile([P, 8, H], F8)
        nc.sync.dma_start(out=keyWT2_sb[:], in_=ch(keyWT28_d[:], 8))

        # phase-B weights: tiles allocated here (resident) but their DMAs
        # are issued after the phase-A enc streams so prologue DMA stays thin
        wihTembG_sb = const.tile([P, 2, FOURH], F8)
        preWTembG_sb = const.tile([P, 2, H], F8)
        preWTh_sb = const.tile([P, 4, H], F16)
        gcT_sb = const.tile([P, 16, NB], FP32)

        # per-batch state, alive for the whole kernel
        state = ctx.enter_context(tc.tile_pool(name="state", bufs=1))
        h0T16_sb = state.tile([P, 4, NB], F16)
        c0T_sb = state.tile([P, 4, NB], FP32)
        qprojR_sb = state.tile([NB, H], F16)
        qprojT_sb = state.tile([P, 4, NB], FP32)
        eTall_sb = state.tile([P, 2, NB], F16)
        ctxT16_sb = state.tile([P, 8, NB], F16)
        rzall_sb = state.tile([NB, 1], FP32)
        gcR_sb = state.tile([NB, FOURH], F16)
        oc_sb = state.tile([NB, H], F16)
        # fp8 coarse+residual pairs for the PSUM fold matmuls (DoubleRow)
        sel8_sb = state.tile([NB, NPAIR * 512], F8)
        gc8p_sb = state.tile([NB, 2, FOURH], F8)
        oc8p_sb = state.tile([NB, 2, H], F8)
        qp8p_sb = state.tile([NB, 2, H], F8)
        embp = ctx.enter_context(tc.tile_pool(name="embp", bufs=NPAIR))

        # ---------------- setup: h0 / c0 / qproj ----------------
        with tc.tile_pool(name="setw", bufs=1) as setw, \
             tc.tile_pool(name="setps", bufs=2, space="PSUM") as setps:
            ehT_sb = setw.tile([P, 8, NB], F16)
            nc.sync.dma_start(out=ehT_sb[:], in_=ch(ehT16_d[:], 8))
            ecT_sb = setw.tile([P, 8, NB], F16)
            nc.sync.dma_start(out=ecT_sb[:], in_=ch(ecT16_d[:], 8))
            bhWT_sb = setw.tile([P, 8, H], F16)
            nc.sync.dma_start(out=bhWT_sb[:], in_=ch(bhWT16_d[:], 8))
            bcWT_sb = setw.tile([P, 8, H], F16)
            nc.sync.dma_start(out=bcWT_sb[:], in_=ch(bcWT16_d[:], 8))
            qWT_sb = setw.tile([P, 4, H], F16)
            nc.sync.dma_start(out=qWT_sb[:], in_=ch(qWT16_d[:], 4))

            for m in range(4):
                ps = setps.tile([P, NB], FP32, tag="ps")
                for k in range(8):
                    nc.tensor.matmul(
                        ps[:], bhWT_sb[:, k, m * P:(m + 1) * P],
                        ehT_sb[:, k, :], start=(k == 0), stop=(k == 7))
                nc.scalar.activation(h0T16_sb[:, m, :], ps[:], AF.Tanh,
                                     bias=hb_sb[:, m:m + 1])
            for m in range(4):
                ps = setps.tile([P, NB], FP32, tag="ps")
                for k in range(8):
                    nc.tensor.matmul(
                        ps[:], bcWT_sb[:, k, m * P:(m + 1) * P],
                        ecT_sb[:, k, :], start=(k == 0), stop=(k == 7))
                nc.scalar.activation(c0T_sb[:, m, :], ps[:], AF.Tanh,
                                     bias=cb_sb[:, m:m + 1])
            # qprojR [b, H] = h0 @ query_W.T  (x16 via qWT16 scaling)
            qps = setps.tile([NB, H], FP32, tag="qps")
            for k in range(4):
                nc.tensor.matmul(qps[:], h0T16_sb[:, k, :], qWT_sb[:, k, :],
                                 start=(k == 0), stop=(k == 3))
            if fold_qproj:
                nc.vector.tensor_copy(qprojR_sb[:], qps[:])
                # pairs stored at 1/16 of the PSUM scale; sel8 carries x16
                nc.vector.tensor_scalar_mul(sel8_sb[:], sel_sb[:], 16.0)
                nc.vector.tensor_scalar_mul(qp8p_sb[:, 0, :], qps[:],
                                            1.0 / 16.0)
                nc.vector.scalar_tensor_tensor(
                    out=qp8p_sb[:, 1, :], in0=qps[:], scalar=1.0 / 16.0,
                    in1=qp8p_sb[:, 0, :], op0=OP.mult, op1=OP.subtract)
            else:
                # transpose to per-(chunk, batch) bias columns, true scale
                qp16 = setw.tile([NB, H], F16)
                nc.vector.tensor_copy(qp16[:], qps[:])
                for m in range(4):
                    tq = setps.tile([P, NB], F16, tag="tq")
                    nc.tensor.transpose(tq[:], qp16[0:NB, m * P:(m + 1) * P],
                                        identity16[0:NB, 0:NB])
                    nc.vector.tensor_scalar_mul(qprojT_sb[:, m, :], tq[:],
                                                1.0 / 16.0)

        # embedding gathers (idx-only dependence; Pool-driven indirect
        # DMAs stream during setup/phase A). Token-major [tok, E].
        emb_tiles = []
        for gi in range(NPAIR):
            ge = embp.tile([P, 4, E], F16, tag="ge")
            for j in range(4):
                nc.gpsimd.indirect_dma_start(
                    out=ge[:, j, :], out_offset=None,
                    in_=emb16_d[:],
                    in_offset=bass.IndirectOffsetOnAxis(
                        ap=idx_sb[:, gi * 4 + j:gi * 4 + j + 1], axis=0))
            emb_tiles.append(ge)
        embT_tiles = []
        embTp = ctx.enter_context(tc.tile_pool(name="embTp", bufs=NPAIR))

        # ---------------- phase A: attention ----------------
        a2w = ctx.enter_context(tc.tile_pool(name="a2w", bufs=1))
        whh_sb = a2w.tile([P, 4, FOURH], F16)
        wcx_sb = a2w.tile([P, 8, FOURH], F16)
        brow_sb = a2w.tile([1, FOURH], F16)
        pwctx_sb = a2w.tile([P, 8, H], F16)
        with tc.tile_pool(name="encTp", bufs=2) as encTp, \
             tc.tile_pool(name="encT2p", bufs=2) as encT2p, \
             tc.tile_pool(name="encp", bufs=3) as encp, \
             tc.tile_pool(name="enerp", bufs=2) as enerp, \
             tc.tile_pool(name="erow", bufs=4) as erow, \
             tc.tile_pool(name="pspk", bufs=3, space="PSUM") as pspk, \
             tc.tile_pool(name="pssc", bufs=1, space="PSUM") as pssc, \
             tc.tile_pool(name="psct", bufs=1, space="PSUM") as psct:
            for i in range(NPAIR if "a" in phases else 0):
                if i == A2W_AT:
                    # A2 weights: queued after the enc streams of this pair
                    nc.sync.dma_start(out=whh_sb[:], in_=ch(whh16_d[:], 4))
                    nc.sync.dma_start(out=wcx_sb[:], in_=ch(wcx16_d[:], 8))
                    nc.sync.dma_start(out=brow_sb[:], in_=brow16_d[:])
                    nc.sync.dma_start(out=pwctx_sb[:],
                                      in_=ch(preWTctx16_d[:], 8))
                encTt = encTp.tile([P, 8, 512], F8, tag="encT")
                nc.sync.dma_start(
                    out=encTt[:],
                    in_=encT8_d[i * P:(i + 1) * P, :].rearrange(
                        "p (k n) -> p k n", k=8))
                if res_both:
                    encT2t = encT2p.tile([P, 8, 512], F8, tag="encT2")
                    nc.sync.dma_start(
                        out=encT2t[:],
                        in_=encT28_d[i * P:(i + 1) * P, :].rearrange(
                            "p (k n) -> p k n", k=8))
                enc_x = []
                for x in range(2):
                    et = encp.tile([P, 2, TWOH], F16, tag="enc")
                    b = 2 * i + x
                    nc.sync.dma_start(
                        out=et[:],
                        in_=enc16_d[b * P:(b + 1) * P, :].rearrange(
                            "p (c n) -> p c n", c=2))
                    enc_x.append(et)

                ener = enerp.tile([P, 4, H], F16, tag="ener")
                for h2 in range(2):
                    pk = pspk.tile([P, TWOH], FP32, tag="pk")
                    for mc in range(2):
                        m = 2 * h2 + mc
                        win = pk[:, mc * H:(mc + 1) * H]
                        mw = slice(m * P, (m + 1) * P)
                        passes = [(keyWT_sb, encTt), (keyWT2_sb, encTt)]
                        if res_both:
                            passes.insert(1, (keyWT_sb, encT2t))
                        for pi, (kw, et) in enumerate(passes):
                            for kp in range(4):
                                kk = slice(2 * kp, 2 * kp + 2)
                                last = (not fold_qproj
                                        and pi == len(passes) - 1
                                        and kp == 3)
                                nc.tensor.matmul(win, kw[:, kk, mw],
                                                 et[:, kk, :],
                                                 start=(pi == 0 and kp == 0),
                                                 stop=last, perf_mode=DR)
                        if fold_qproj:
                            s8 = sel8_sb[0:NB, i * 512:(i + 1) * 512]
                            s8p = bass.AP(s8.tensor, s8.offset,
                                          [s8.ap[0], [0, 2], s8.ap[1]])
                            nc.tensor.matmul(
                                win, qp8p_sb[0:NB, 0:2, mw], s8p,
                                start=False, stop=True, perf_mode=DR)
                    if fold_qproj:
                        nc.scalar.activation(
                            ener[:, 2 * h2:2 * h2 + 2, :],
                            pk[:].rearrange("p (c n) -> p c n", c=2),
                            AF.Tanh, scale=1.0 / 16.0)
                    else:
                        for mc in range(2):
                            m = 2 * h2 + mc
                            for x in range(2):
                                b = 2 * i + x
                                nc.scalar.activation(
                                    ener[:, m, x * S:(x + 1) * S],
                                    pk[:, mc * H + x * S:mc * H + (x + 1) * S],
                                    AF.Tanh, scale=1.0 / 16.0,
                                    bias=qprojT_sb[:, m, b:b + 1])

                ctall = psct.tile([P, 20], FP32, tag="ct")
                for x in range(2):
                    sc = pssc.tile([1, S], FP32, tag="sc")
                    for m in range(4):
                        nc.tensor.matmul(
                            sc[:], energyW_sb[:, m:m + 1],
                            ener[:, m, x * S:(x + 1) * S],
                            start=(m == 0), stop=(m == 3))
                    # scores are bounded (|s| < ~6) for this model: softmax
                    # runs unstabilized, unnormalized.
                    e16 = erow.tile([1, S], F16, tag="e16")
                    nc.scalar.activation(e16[:], sc[:], AF.Exp,
                                         scale=1.0 / 16.0)
                    for c in range(2):
                        nc.tensor.matmul(ctall[:, c * 2 + x:c * 2 + x + 1],
                                         e16[0:1, c * P:(c + 1) * P],
                                         ones16f[0:1, 0:1], start=True,
                                         stop=True)
                nc.vector.tensor_copy(
                    eTall_sb[:, :, 2 * i:2 * i + 2],
                    ctall[:, 0:4].rearrange("p (c x) -> p c x", c=2))
                for x in range(2):
                    b = 2 * i + x
                    for c8 in range(8):
                        for c in range(2):
                            nc.tensor.matmul(
                                ctall[:, 4 + c8 * 2 + x:5 + c8 * 2 + x],
                                enc_x[x][:, c, c8 * P:(c8 + 1) * P],
                                eTall_sb[:, c, b:b + 1],
                                start=(c == 0), stop=(c == 1))
                # ctxT16 = sum(e * enc) / 16 = Z*ctx/16 (unnormalized Z~500)
                nc.vector.tensor_scalar_mul(
                    ctxT16_sb[:, :, 2 * i:2 * i + 2],
                    ctall[:, 4:20].rearrange("p (c x) -> p c x", c=8),
                    1.0 / 16.0)


        if "a" not in phases:
            nc.vector.memset(eTall_sb[:], 0.5)
            nc.vector.memset(ctxT16_sb[:], 0.5)
            nc.sync.dma_start(out=whh_sb[:], in_=ch(whh16_d[:], 4))
            nc.sync.dma_start(out=wcx_sb[:], in_=ch(wcx16_d[:], 8))
            nc.sync.dma_start(out=brow_sb[:], in_=brow16_d[:])
            nc.sync.dma_start(out=pwctx_sb[:], in_=ch(preWTctx16_d[:], 8))

        # transpose gathered [tok, E] -> DR layout [e%128, e//128, tok];
        # runs on the PE during the A2 weight-DMA window
        with tc.tile_pool(name="pstr", bufs=2, space="PSUM") as pstr:
            for ti in range(NPAIR):
                embT = embTp.tile([P, 2, 512], F8, tag="embT")
                tp = pstr.tile([P, 2, 512], F16, tag="tp")
                for j in range(4):
                    for e2 in range(2):
                        nc.tensor.transpose(
                            tp[:, e2, j * P:(j + 1) * P],
                            emb_tiles[ti][:, j, e2 * P:(e2 + 1) * P],
                            identity16[:])
                nc.scalar.copy(embT[:].rearrange("p c n -> p (c n)"),
                               tp[:].rearrange("p c n -> p (c n)"))
                embT_tiles.append(embT)

        # ---------------- A2: fold 1/Z, per-batch constants ----------------
        with tc.tile_pool(name="psa2", bufs=1, space="PSUM") as psa2, \
             tc.tile_pool(name="psgc", bufs=1, space="PSUM") as psgc:

            zps = psa2.tile([NB, 1], FP32, tag="z")
            for c in range(2):
                nc.tensor.matmul(zps[:], eTall_sb[:, c, :], onescol16[:, 0:1],
                                 start=(c == 0), stop=(c == 1))
            nc.vector.reciprocal(rzall_sb[:], zps[:])  # = 256 / sum(e)

            for half in range(2):
                gch = psgc.tile([NB, FOURH // 2], FP32, tag="gch")
                gcx = psgc.tile([NB, FOURH // 2], FP32, tag="gcx")
                gch16 = a2w.tile([NB, FOURH // 2], F16, tag="gch16")
                for nw2 in range(2):
                    nw = 2 * half + nw2
                    win = slice(nw * H, (nw + 1) * H)
                    pw = slice(nw2 * H, (nw2 + 1) * H)
                    for k in range(4):
                        nc.tensor.matmul(gch[:, pw], h0T16_sb[:, k, :],
                                         whh_sb[:, k, win],
                                         start=(k == 0), stop=False)
                    nc.tensor.matmul(gch[:, pw], ones16f[0:1, :],
                                     brow_sb[0:1, win], start=False, stop=True)
                    for k in range(8):
                        nc.tensor.matmul(gcx[:, pw], ctxT16_sb[:, k, :],
                                         wcx_sb[:, k, win],
                                         start=(k == 0), stop=(k == 7))
                nc.vector.tensor_copy(gch16[:], gch[:])
                nc.vector.scalar_tensor_tensor(
                    out=gcR_sb[:, half * TWOH:(half + 1) * TWOH],
                    in0=gcx[:], scalar=rzall_sb[:, 0:1],
                    in1=gch16[:], op0=OP.mult, op1=OP.add)

            ocps = psa2.tile([NB, H], FP32, tag="oc")
            for k in range(8):
                nc.tensor.matmul(ocps[:], ctxT16_sb[:, k, :], pwctx_sb[:, k, :],
                                 start=(k == 0), stop=(k == 7))
            nc.vector.tensor_scalar_mul(oc_sb[:], ocps[:], rzall_sb[:, 0:1])
            if fold_gates:
                nc.vector.tensor_scalar_mul(gc8p_sb[:, 0, :], gcR_sb[:],
                                            1.0 / 16.0)
                nc.vector.scalar_tensor_tensor(
                    out=gc8p_sb[:, 1, :], in0=gcR_sb[:], scalar=1.0 / 16.0,
                    in1=gc8p_sb[:, 0, :], op0=OP.mult, op1=OP.subtract)
            nc.vector.tensor_copy(oc8p_sb[:, 0, :], oc_sb[:])
            nc.vector.tensor_tensor(out=oc8p_sb[:, 1, :], in0=oc_sb[:],
                                    in1=oc8p_sb[:, 0, :], op=OP.subtract)

            if not fold_gates:
                # transpose gcR rows into per-(chunk, batch) bias columns,
                # true scale (gcR is x256)
                for mg in range(16):
                    tg = psa2.tile([P, NB], F16, tag="tg")
                    nc.tensor.transpose(
                        tg[:], gcR_sb[0:NB, mg * P:(mg + 1) * P],
                        identity16[0:NB, 0:NB])
                    nc.vector.tensor_scalar_mul(gcT_sb[:, mg, :], tg[:],
                                                1.0 / 256.0)

            # phase-B weights (DMAs deferred here so the phase-A enc
            # streams own the DMA engines during phase A)
            nc.sync.dma_start(out=wihTembG_sb[:], in_=ch(wihTembG8_d[:], 2))
            nc.sync.dma_start(out=preWTembG_sb[:], in_=ch(preWTembG8_d[:], 2))
            nc.sync.dma_start(out=preWTh_sb[:], in_=ch(preWTh16_d[:], 4))

        # ---------------- phase B: gates, LSTM, projection ----------------
        # wave order: i, g, f, o (mg = gate*4 + hs; torch order i,f,g,o)
        WAVES = [(0, AF.Sigmoid), (2, AF.Tanh), (1, AF.Sigmoid),
                 (3, AF.Sigmoid)]
        with tc.tile_pool(name="gmega", bufs=2) as gmega, \
             tc.tile_pool(name="lstm", bufs=2) as lstm, \
             tc.tile_pool(name="outp", bufs=2) as outp, \
             tc.tile_pool(name="pswv", bufs=3, space="PSUM") as pswv, \
             tc.tile_pool(name="pspo", bufs=2, space="PSUM") as pspo:
            pend = None
            for i in range((NPAIR + 1) if "b" in phases else 0):
              if i < NPAIR:
                embT = embT_tiles[i]
                mega = {}
                for g, fn in WAVES:
                    dst = gmega.tile([P, 4, 512], F16, tag=f"m{g}")
                    mega[g] = dst
                    for hp in range(2):
                        wv = pswv.tile([P, TWOH], FP32, tag="wv")
                        for mc in range(2):
                            hs = 2 * hp + mc
                            mg = g * 4 + hs
                            win = wv[:, mc * H:(mc + 1) * H]
                            nc.tensor.matmul(
                                win, wihTembG_sb[:, 0:2, mg * P:(mg + 1) * P],
                                embT[:, 0:2, :], start=True,
                                stop=not fold_gates, perf_mode=DR)
                            if fold_gates:
                                s8 = sel8_sb[0:NB, i * 512:(i + 1) * 512]
                                s8p = bass.AP(s8.tensor, s8.offset,
                                              [s8.ap[0], [0, 2], s8.ap[1]])
                                nc.tensor.matmul(
                                    win,
                                    gc8p_sb[0:NB, 0:2, mg * P:(mg + 1) * P],
                                    s8p, start=False, stop=True, perf_mode=DR)
                        if fold_gates:
                            nc.scalar.activation(
                                dst[:, 2 * hp:2 * hp + 2, :],
                                wv[:].rearrange("p (c n) -> p c n", c=2),
                                fn, scale=1.0 / 256.0)
                        else:
                            for mc in range(2):
                                hs = 2 * hp + mc
                                mg = g * 4 + hs
                                for x in range(2):
                                    b = 2 * i + x
                                    nc.scalar.activation(
                                        dst[:, hs, x * S:(x + 1) * S],
                                        wv[:, mc * H + x * S:
                                           mc * H + (x + 1) * S],
                                        fn, scale=1.0 / 256.0,
                                        bias=gcT_sb[:, mg, b:b + 1])

                sI, tG, sF, sO = mega[0], mega[2], mega[1], mega[3]
                tmp = lstm.tile([P, FOURH], F16, tag="tmp")
                nc.vector.tensor_tensor(out=tmp[:],
                                        in0=sI[:].rearrange("p c n -> p (c n)"),
                                        in1=tG[:].rearrange("p c n -> p (c n)"),
                                        op=OP.mult)
                cc = lstm.tile([P, 4, 2, S], F16, tag="cc")
                for hs in range(4):
                    for x in range(2):
                        b = 2 * i + x
                        nc.vector.scalar_tensor_tensor(
                            out=cc[:, hs, x, :],
                            in0=sF[:, hs, x * S:(x + 1) * S],
                            scalar=c0T_sb[:, hs, b:b + 1],
                            in1=tmp[:, hs * 512 + x * S:
                                    hs * 512 + (x + 1) * S],
                            op0=OP.mult, op1=OP.add)
                tc16 = lstm.tile([P, FOURH], F16, tag="tc")
                nc.scalar.activation(tc16[:],
                                     cc[:].rearrange("p h x s -> p (h x s)"),
                                     AF.Tanh)
                h16 = lstm.tile([P, 4, 512], F16, tag="h16")
                nc.vector.scalar_tensor_tensor(
                    out=h16[:].rearrange("p c n -> p (c n)"),
                    in0=sO[:].rearrange("p c n -> p (c n)"), scalar=16.0,
                    in1=tc16[:], op0=OP.mult, op1=OP.mult)

                pend_new = (i, embT, h16)
              else:
                pend_new = None
              if pend is not None:
                pi_, embT_, h16_ = pend
                for tci in range(4):
                    x, tokw = tci // 2, tci % 2
                    b = 2 * pi_ + x
                    tw = slice(x * S + tokw * P, x * S + (tokw + 1) * P)
                    po = pspo.tile([P, H], FP32, tag="po")
                    nc.tensor.matmul(po[:],
                                     embT_[:, 0:2, tci * P:(tci + 1) * P],
                                     preWTembG_sb[:, 0:2, :],
                                     start=True, stop=False, perf_mode=DR)
                    for hs in range(4):
                        nc.tensor.matmul(po[:], h16_[:, hs, tw],
                                         preWTh_sb[:, hs, :],
                                         start=False, stop=False)
                    idc = identity8[0:NB, b:b + 1]
                    idp = bass.AP(idc.tensor, idc.offset,
                                  [idc.ap[0], [0, 2], [0, P]])
                    nc.tensor.matmul(po[:], idp, oc8p_sb[0:NB, 0:2, :],
                                     start=False, stop=True, perf_mode=DR)
                    o16 = outp.tile([P, H], F16, tag="o16")
                    nc.vector.tensor_copy(o16[:], po[:])
                    nc.sync.dma_start(
                        out=out_d[pi_ * 512 + tci * P:
                                  pi_ * 512 + (tci + 1) * P, :],
                        in_=o16[:])
              pend = pend_new
    return nc


# ---------------------------------------------------------------------------
# host side
# ---------------------------------------------------------------------------

def _to8(x):
    return np.clip(np.asarray(x, np.float32), -240.0, 240.0).astype(f8np)


def _chunkmajor(v, chunks):
    return np.ascontiguousarray(v.reshape(chunks, P).T).astype(np.float32)


def prep_inputs(inputs, n_cores=N_CORES):
    f32 = lambda x: np.asarray(x, dtype=np.float32)
    f16c = lambda x: np.ascontiguousarray(f32(x)).astype(np.float16)
    tgt = np.asarray(inputs["tgt_seq"]).astype(np.int32)
    enc = f32(inputs["encoder_output"])
    eh = f32(inputs["encoder_hidden"])[0]
    ec = f32(inputs["encoder_cell"])[0]
    W_ih = f32(inputs["W_ih"])
    W_hh = f32(inputs["W_hh"])
    pre_W = f32(inputs["pre_W"])
    emb = f32(inputs["emb"])

    sel = np.zeros((NB, NPAIR, 2, 256), np.float16)
    for i in range(NPAIR):
        sel[2 * i, i, 0, :] = 1.0
        sel[2 * i + 1, i, 1, :] = 1.0

    k16 = f32(inputs["key_W"]).T * 16.0
    k1 = _to8(k16)
    k2 = _to8(k16 - k1.astype(np.float32))

    shared = dict(
        emb16=np.ascontiguousarray(emb * 16.0).astype(np.float16),
        sel16=np.ascontiguousarray(sel.reshape(NB, NPAIR * 512)),
        bhWT16=f16c(f32(inputs["bridge_hW"]).T),
        bcWT16=f16c(f32(inputs["bridge_cW"]).T),
        hb32=_chunkmajor(f32(inputs["bridge_hb"]), 4),
        cb32=_chunkmajor(f32(inputs["bridge_cb"]), 4),
        qWT16=f16c(f32(inputs["query_W"]).T * 16.0),
        keyWT8=k1,
        keyWT28=k2,
        energyW16=_chunkmajor(f32(inputs["energy_W"])[0], 4).astype(
            np.float16) * np.float16(16.0),
        whh16=f16c(W_hh.T * 256.0),
        wcx16=f16c(W_ih[:, E:].T * 16.0),
        brow16=f16c((f32(inputs["b_ih"]) + f32(inputs["b_hh"]))[None, :]
                    * 256.0),
        wihTembG8=_to8(W_ih[:, :E].T * 16.0),
        preWTembG8=_to8(pre_W[:, :E].T * 16.0),
        preWTh16=f16c(pre_W[:, E:E + H].T * 16.0),
        preWTctx16=f16c(pre_W[:, E + H:].T * 16.0),
    )

    in_maps = []
    for ci in range(n_cores):
        sl = slice(ci * NB, (ci + 1) * NB)
        enc_c = enc[sl]                                    # [NB, S, 2H]
        # encT8 [pair*128+p, k*512 + b*256 + s] = enc[2i+b, s, k*128+p]
        encT = enc_c.reshape(NPAIR, 2, S, 8, P)
        encT = np.transpose(encT, (0, 4, 3, 1, 2))          # [i, p, k, b, s]
        encT = np.ascontiguousarray(encT.reshape(NPAIR * P, 8 * 512))
        encT1 = _to8(encT)
        encT2 = _to8(encT - encT1.astype(np.float32))
        # enc16 [b*128+p, c*1024+d] = enc[b, c*128+p, d]
        enc16 = enc_c.reshape(NB, 2, P, TWOH)
        enc16 = np.transpose(enc16, (0, 2, 1, 3))           # [b, p, c, d]

        tgtc = tgt[sl]
        idx = np.zeros((P, NPAIR * 4), np.int32)
        for i in range(NPAIR):
            flat = tgtc[2 * i:2 * i + 2].reshape(512)
            idx[:, i * 4:(i + 1) * 4] = flat.reshape(4, P).T

        in_maps.append(dict(
            encT8=encT1,
            encT28=encT2,
            enc16=enc16.reshape(NB * P, 2 * TWOH).astype(np.float16),
            idx32=idx,
            ehT16=f16c(eh[sl].T),
            ecT16=f16c(ec[sl].T),
            **shared,
        ))
    return in_maps, NB


_CACHED = {}


FOLD_GATES = True
FOLD_QPROJ = True
RES_BOTH = False


PHASES = "ab"


def _get_nc(key=0, iters=1):
    k = (key, iters, FOLD_GATES, FOLD_QPROJ, RES_BOTH, PHASES)
    if k not in _CACHED:
        nc = bacc.Bacc("TRN2", target_bir_lowering=False, debug=False)
        build_kernel(nc, iters=iters, fold_gates=FOLD_GATES,
                     fold_qproj=FOLD_QPROJ, res_both=RES_BOTH, phases=PHASES)
        nc.compile()
        _CACHED[k] = nc
    return _CACHED[k]


def kernel(**inputs):
    in_maps, _ = prep_inputs(inputs, N_CORES)
    nc = _get_nc()
    res = run_bass_kernel_spmd(nc, in_maps, list(range(N_CORES)))
    B = np.asarray(inputs["tgt_seq"]).shape[0]
    out = np.empty((B, T, H), dtype=np.float32)
    for i in range(N_CORES):
        o = res.results[i]["out"].astype(np.float32) / 256.0
        out[i * NB:(i + 1) * NB] = o.reshape(NB, T, H)
    return out
